# revision 3
# baseline (speedup 1.0000x reference)
"""Causal multi-head attention (B=4, T=2048, C=1024, H=16) on 8 TRN2 cores.

Sharding: batch (4) x head-group (2 groups of 8 heads) -> 8 shards, one per
core. Each core computes QKV projections for its 8 heads, causal flash-style
attention, and a Megatron row-parallel slice of the output projection; the
host sums the two head-group partial outputs per batch element.

v6: all matmuls bf16 (fp32 psum accum). Attention interleaves the two heads
of each pair so consecutive S matmuls occupy disjoint PE row groups (rows
0-63 / 64-127, K=64) and run concurrently. The Q/K projections for head pair
hp+1 are emitted as parcels inside attention(hp)'s loop, filling the PE while
the ACT engine grinds exp - a dedicated 1-bank psum pool (freed by moving
the softmax-denominator broadcast from a PE matmul to gpsimd
partition_broadcast) feeds them. K stays resident in SBUF (no DRAM spill).
Diagonal staircase packs to 1280 columns; the 256-wide j2 block uses a
per-head bufs=1 psum tile so the two heads' row-group-concurrent matmuls
never drain into one bank simultaneously (that crashes the device).
Phase-3 bias rides ACT.

Self-contained: hardcodes shapes from the problem spec; no file reads.
"""
import sys
sys.path.insert(0, '/opt/trn_rl_repo')
import numpy as np

B, T, C = 4, 2048, 1024
H, D = 16, 64
N_CORES = 8
HPC = 8        # heads per core
HP = 4         # head pairs per core
KB = 16        # 128-row key tiles per sequence
NQSB = 4       # 512-column query superblocks
CI = 8         # 128-row contraction tiles over C
VW = 66        # V_aug stride per head (64 V + 1 ones + 1 pad)

QOFF = (0, 128, 256, 384)
POFF = (0, 512, 1024, 896)
PTW = 1280     # packed staircase width (j0 512 + j1 384 + j3 128 + j2 256)

_CACHE = {}


def build_nc(iters=1):
    import contextlib
    import concourse.tile as tile
    from concourse import bacc, mybir

    F32 = mybir.dt.float32
    BF16 = mybir.dt.bfloat16
    EXP = mybir.ActivationFunctionType.Exp
    IDENT = mybir.ActivationFunctionType.Identity

    nc = bacc.Bacc("TRN2", target_bir_lowering=False, debug=False)

    xT_d = nc.dram_tensor("xT", [C, T], BF16, kind="ExternalInput")
    wqT_d = nc.dram_tensor("wqT", [C, 512], BF16, kind="ExternalInput")
    wkT_d = nc.dram_tensor("wkT", [C, 512], BF16, kind="ExternalInput")
    wvT_d = nc.dram_tensor("wvT", [C, 512], BF16, kind="ExternalInput")
    woT_d = nc.dram_tensor("woT", [512, C], BF16, kind="ExternalInput")
    bias_d = nc.dram_tensor("bias", [128, 8], F32, kind="ExternalInput")
    mask_d = nc.dram_tensor("masks", [128, PTW], BF16, kind="ExternalInput")
    yT_d = nc.dram_tensor("yT", [C, T], F32, kind="ExternalOutput")

    with tile.TileContext(nc) as tc:
        def emit():
            with contextlib.ExitStack() as es:
                const = es.enter_context(tc.tile_pool(name="const", bufs=1))
                qtp = es.enter_context(tc.tile_pool(name="qt", bufs=1))
                ctxp = es.enter_context(tc.tile_pool(name="ctx", bufs=1))
                ktp = es.enter_context(tc.tile_pool(name="ktp", bufs=1))
                vp = es.enter_context(tc.tile_pool(name="vsb", bufs=1))
                xtp = es.enter_context(tc.tile_pool(name="xt", bufs=1))
                wqp = es.enter_context(tc.tile_pool(name="wq", bufs=2))
                wkp = es.enter_context(tc.tile_pool(name="wk", bufs=2))
                maskp = es.enter_context(tc.tile_pool(name="maskp", bufs=1))
                wop = es.enter_context(tc.tile_pool(name="wo", bufs=1))
                qkp = es.enter_context(
                    tc.tile_pool(name="qkp", bufs=1, space="PSUM"))

                ones_f = const.tile([128, 64], F32)
                nc.any.memset(ones_f[:], 1.0)
                ones_r = const.tile([128, 64], BF16)
                nc.vector.tensor_copy(ones_r[:], ones_f[:])
                ones16_f = const.tile([128, 16], F32)
                nc.any.memset(ones16_f[:], 1.0)
                ones16_r = const.tile([128, 16], BF16)
                nc.vector.tensor_copy(ones16_r[:], ones16_f[:])
                bias_sb = const.tile([128, 8], F32)
                nc.sync.dma_start(bias_sb[:], bias_d.ap())
                mask_sb = maskp.tile([128, PTW], BF16)
                nc.sync.dma_start(mask_sb[:], mask_d.ap())

                qt_sb, ctx_sb, kt_sb, v_sb, wo_sb = [], [], [], [], []
                for hp in range(HP):
                    qt_sb.append(qtp.tile([128, T], BF16, tag=f"qt{hp}",
                                          name=f"qt{hp}"))
                    ctx_sb.append(ctxp.tile([128, T], BF16, tag=f"ctx{hp}",
                                            name=f"ctx{hp}"))
                    kt_sb.append(ktp.tile([128, T], BF16, tag=f"kt{hp}",
                                          name=f"kt{hp}"))
                    w_ = wop.tile([128, C], BF16, tag=f"wo{hp}",
                                  name=f"wo{hp}")
                    nc.sync.dma_start(
                        w_[:], woT_d.ap()[hp * 128:(hp + 1) * 128, :])
                    wo_sb.append(w_)
                for kb in range(KB):
                    v_sb.append(vp.tile([128, HPC * VW], BF16, tag=f"v{kb}",
                                        name=f"v{kb}"))

                xt_sb = []
                for ci in range(CI):
                    t_ = xtp.tile([128, T], BF16, tag=f"xt{ci}")
                    nc.sync.dma_start(t_[:],
                                      xT_d.ap()[ci * 128:(ci + 1) * 128, :])
                    xt_sb.append(t_)

                wq_sb = [None] * HP
                wk_sb = [None] * HP

                def load_weights(hp):
                    fsl = slice(hp * 128, (hp + 1) * 128)
                    wq_sb[hp], wk_sb[hp] = [], []
                    for ci in range(CI):
                        tq = wqp.tile([128, 128], BF16, tag=f"wqs{ci}",
                                      name="wqci")
                        nc.sync.dma_start(
                            tq[:], wqT_d.ap()[ci * 128:(ci + 1) * 128, fsl])
                        wq_sb[hp].append(tq)
                        tk = wkp.tile([128, 128], BF16, tag=f"wks{ci}",
                                      name="wkci")
                        nc.sync.dma_start(
                            tk[:], wkT_d.ap()[ci * 128:(ci + 1) * 128, fsl])
                        wk_sb[hp].append(tk)

                def proj_parcels(hp):
                    """Generator: 8 parcels, each one Q or K projection
                    accumulation group (8 matmuls + DVE copy-out)."""
                    for tj in range(NQSB):
                        for which in range(2):
                            yield (hp, tj, which)

                def emit_parcel(p):
                    hp, tj, which = p
                    tsl = slice(tj * 512, (tj + 1) * 512)
                    w_sb = wq_sb[hp] if which == 0 else wk_sb[hp]
                    dst = qt_sb[hp] if which == 0 else kt_sb[hp]
                    ps_ = qkp.tile([128, 512], F32, tag="qk", name="qk")
                    for ci in range(CI):
                        nc.tensor.matmul(
                            ps_[:], w_sb[ci][:], xt_sb[ci][:, tsl],
                            start=(ci == 0), stop=(ci == CI - 1),
                            skip_group_check=True)
                    nc.vector.tensor_copy(dst[:, tsl], ps_[:])

                # ---------------- phase 1a: V projections ----------------
                with contextlib.ExitStack() as p1a:
                    wvp = p1a.enter_context(tc.tile_pool(name="wv", bufs=1))
                    vps = p1a.enter_context(
                        tc.tile_pool(name="vps", bufs=4, space="PSUM"))
                    wv_sb = []
                    for ci in range(CI):
                        t_ = wvp.tile([128, 512], BF16, tag=f"wv{ci}")
                        nc.sync.dma_start(
                            t_[:], wvT_d.ap()[ci * 128:(ci + 1) * 128, :])
                        wv_sb.append(t_)
                    for ti in range(KB):
                        ps_ = vps.tile([128, 512], F32)
                        for ci in range(CI):
                            nc.tensor.matmul(
                                ps_[:],
                                xt_sb[ci][:, ti * 128:(ti + 1) * 128],
                                wv_sb[ci][:],
                                start=(ci == 0), stop=(ci == CI - 1),
                                skip_group_check=True)
                        sv = v_sb[ti][:].rearrange("p (h w) -> p h w", w=VW)
                        nc.vector.tensor_copy(
                            sv[:, :, 64:66],
                            ones16_r[:].rearrange("p (h w) -> p h w", w=2))
                        nc.vector.tensor_copy(
                            sv[:, :, 0:64],
                            ps_[:].rearrange("p (h w) -> p h w", w=64))

                # -------- projections for head pair 0 (sequential) --------
                load_weights(0)
                load_weights(1)
                for p in proj_parcels(0):
                    emit_parcel(p)

                # ---------------- phase 2: attention ----------------
                with contextlib.ExitStack() as p2:
                    ptp = p2.enter_context(tc.tile_pool(name="pt", bufs=4))
                    rrp = p2.enter_context(tc.tile_pool(name="rr", bufs=2))
                    rawp = p2.enter_context(tc.tile_pool(name="raw",
                                                          bufs=3))
                    tmpp = p2.enter_context(tc.tile_pool(name="tmp", bufs=2))
                    sps0 = p2.enter_context(
                        tc.tile_pool(name="sps0", bufs=1, space="PSUM"))
                    sps1 = p2.enter_context(
                        tc.tile_pool(name="sps1", bufs=1, space="PSUM"))
                    spbp = p2.enter_context(
                        tc.tile_pool(name="spb", bufs=1, space="PSUM"))
                    pvps0 = p2.enter_context(
                        tc.tile_pool(name="pvps0", bufs=1, space="PSUM"))
                    pvps1 = p2.enter_context(
                        tc.tile_pool(name="pvps1", bufs=1, space="PSUM"))

                    PSL = (slice(0, 64), slice(64, 128))
                    for hp in range(HP):
                        kt, qt = kt_sb[hp], qt_sb[hp]
                        if hp + 1 < HP:
                            if hp + 2 < HP:
                                load_weights(hp + 2)
                            parcels = proj_parcels(hp + 1)
                        else:
                            parcels = iter(())

                        def fill():
                            p = next(parcels, None)
                            if p is not None:
                                emit_parcel(p)

                        for qsb in range(NQSB):
                            qbase = qsb * 512
                            qsl = slice(qbase, qbase + 512)
                            n_full = 4 * qsb
                            vsl = [slice((2 * hp + hl) * VW,
                                         (2 * hp + hl) * VW + 65)
                                   for hl in range(2)]
                            pv = [pvps0.tile([128, 512], F32, tag="pv0",
                                             name="pv0"),
                                  pvps1.tile([128, 512], F32, tag="pv1",
                                             name="pv1")]
                            first = [True, True]
                            for kbp in range(n_full // 2):
                                kb0, kb1 = 2 * kbp, 2 * kbp + 1
                                sp = [sps0.tile([128, 1024], F32, tag="sp0",
                                                name="sp0"),
                                      sps1.tile([128, 1024], F32, tag="sp1",
                                                name="sp1")]
                                for u, kb in enumerate((kb0, kb1)):
                                    for hl in range(2):
                                        nc.tensor.matmul(
                                            sp[hl][:, u * 512:(u + 1) * 512],
                                            kt[PSL[hl],
                                               kb * 128:(kb + 1) * 128],
                                            qt[PSL[hl], qsl],
                                            start=True, stop=True,
                                            skip_group_check=True)
                                fill()
                                pt = [None, None]
                                for hl in range(2):
                                    pt[hl] = ptp.tile([128, PTW], BF16,
                                                      tag="pt", name="pt")
                                    nc.scalar.activation(
                                        pt[hl][:, 0:1024], sp[hl][:],
                                        EXP, scale=0.125)
                                for u, kb in enumerate((kb0, kb1)):
                                    for hl in range(2):
                                        nc.tensor.matmul(
                                            pv[hl][0:65, :],
                                            v_sb[kb][:, vsl[hl]],
                                            pt[hl][:, u * 512:(u + 1) * 512],
                                            start=first[hl], stop=False,
                                            skip_group_check=True)
                                        first[hl] = False
                            # diagonal staircase
                            sp_a = [sps0.tile([128, 1024], F32, tag="sp0",
                                              name="spa0"),
                                    sps1.tile([128, 1024], F32, tag="sp1",
                                              name="spa1")]
                            for j in (0, 1, 3):
                                kb = n_full + j
                                n_ = 512 - QOFF[j]
                                for hl in range(2):
                                    nc.tensor.matmul(
                                        sp_a[hl][:, POFF[j]:POFF[j] + n_],
                                        kt[PSL[hl],
                                           kb * 128:(kb + 1) * 128],
                                        qt[PSL[hl],
                                           qbase + QOFF[j]:qbase + 512],
                                        start=True, stop=True,
                                        skip_group_check=True)
                            fill()
                            pt = [None, None]
                            for hl in range(2):
                                # per-head spb (bufs=1, same tag): head 1's
                                # write serializes behind head 0's exp read,
                                # so two row-group-concurrent matmuls never
                                # drain into this bank at the same time
                                spb = spbp.tile([128, 256], F32, tag="spb",
                                                name="spb")
                                kb2 = n_full + 2
                                nc.tensor.matmul(
                                    spb[:],
                                    kt[PSL[hl], kb2 * 128:(kb2 + 1) * 128],
                                    qt[PSL[hl],
                                       qbase + QOFF[2]:qbase + 512],
                                    start=True, stop=True,
                                    skip_group_check=True)
                                p_ = ptp.tile([128, PTW], BF16, tag="pt",
                                              name="pt")
                                nc.scalar.activation(p_[:, 0:1024],
                                                     sp_a[hl][:],
                                                     EXP, scale=0.125)
                                nc.scalar.activation(p_[:, 1024:PTW],
                                                     spb[:],
                                                     EXP, scale=0.125)
                                nc.vector.tensor_mul(p_[:], p_[:],
                                                     mask_sb[:])
                                pt[hl] = p_
                                for j in (0, 1, 3, 2):
                                    kb = n_full + j
                                    n_ = 512 - QOFF[j]
                                    nc.tensor.matmul(
                                        pv[hl][0:65, QOFF[j]:512],
                                        v_sb[kb][:, vsl[hl]],
                                        pt[hl][:, POFF[j]:POFF[j] + n_],
                                        start=first[hl], stop=(j == 2),
                                        skip_group_check=True)
                                    first[hl] = False
                            # normalize: ctx = pv[0:64] / pv[64]; denominator
                            # reciprocal broadcast across partitions on gpsimd
                            for hl in range(2):
                                rr = rrp.tile([65, 512], BF16, tag="rr",
                                              name="rr")
                                with nc.allow_low_precision("softmax denom"):
                                    nc.vector.reciprocal(rr[64:65, :],
                                                         pv[hl][64:65, :])
                                # denominator broadcast via PE, time-sharing
                                # the projection-parcel psum bank
                                bc = qkp.tile([128, 512], F32, tag="qk",
                                              name="bc")
                                nc.tensor.matmul(bc[0:64, :],
                                                 ones_r[64:65, :],
                                                 rr[64:65, :],
                                                 start=True, stop=True,
                                                 skip_group_check=True)
                                raw = rawp.tile([64, 512], F32, tag="raw",
                                                name="raw")
                                nc.vector.tensor_copy(raw[:],
                                                      pv[hl][0:64, :])
                                if hl == 0:
                                    nc.vector.tensor_mul(
                                        ctx_sb[hp][0:64, qsl],
                                        raw[:], bc[0:64, :])
                                else:
                                    tmp = tmpp.tile([64, 512], BF16,
                                                    tag="tmp", name="tmp")
                                    nc.vector.tensor_mul(tmp[:], raw[:],
                                                         bc[0:64, :])
                                    nc.sync.dma_start(
                                        ctx_sb[hp][64:128, qsl], tmp[:])
                        for p in parcels:
                            emit_parcel(p)

                    # -------------- phase 3: output projection --------------
                    with contextlib.ExitStack() as p3:
                        yp = p3.enter_context(tc.tile_pool(name="y", bufs=3))
                        for oi in range(8):
                            osl = slice(oi * 128, (oi + 1) * 128)
                            for tj in range(NQSB):
                                tsl = slice(tj * 512, (tj + 1) * 512)
                                yps = pvps0 if (oi * NQSB + tj) % 2 == 0 \
                                    else pvps1
                                ps_ = yps.tile([128, 512], F32,
                                               tag="pv0" if yps is pvps0
                                               else "pv1", name="yacc")
                                for hp in range(HP):
                                    nc.tensor.matmul(
                                        ps_[:], wo_sb[hp][:, osl],
                                        ctx_sb[hp][:, tsl],
                                        start=(hp == 0), stop=(hp == HP - 1),
                                        skip_group_check=True)
                                y_ = yp.tile([128, 512], F32)
                                nc.scalar.activation(
                                    y_[:], ps_[:], IDENT,
                                    bias=bias_sb[:, oi:oi + 1])
                                nc.sync.dma_start(yT_d.ap()[osl, tsl], y_[:])

        if iters == 1:
            emit()
        else:
            with tc.For_i(0, iters, 1):
                emit()
    nc.compile()
    return nc


def make_masks():
    """Packed staircase mask [128, PTW]: pt col POFF[j] + (q - QOFF[j])
    holds causal keep-bit for key row k = 128*j + k_local vs query q."""
    m = np.zeros((128, PTW), np.float32)
    k = np.arange(128)[:, None]
    for j in range(4):
        q = np.arange(QOFF[j], 512)[None, :]
        m[:, POFF[j]:POFF[j] + 512 - QOFF[j]] = (q >= 128 * j + k)
    return m


def shard_inputs(x, w_qkv, w_out, b_out):
    """Full inputs -> list of 8 per-core input dicts."""
    import ml_dtypes
    bf16 = ml_dtypes.bfloat16
    x = np.asarray(x, dtype=np.float32).astype(bf16)
    w_qkv = np.asarray(w_qkv, dtype=np.float32).astype(bf16)
    w_out = np.asarray(w_out, dtype=np.float32).astype(bf16)
    b_out = np.asarray(b_out, dtype=np.float32)
    masks = make_masks().astype(bf16)
    in_maps = []
    for c in range(N_CORES):
        b, hg = c // 2, c % 2
        h0 = hg * HPC
        csl = slice(h0 * D, (h0 + HPC) * D)
        im = {
            "xT": np.ascontiguousarray(x[b].T),
            "wqT": np.ascontiguousarray(w_qkv[0 * C:1 * C][csl].T),
            "wkT": np.ascontiguousarray(w_qkv[1 * C:2 * C][csl].T),
            "wvT": np.ascontiguousarray(w_qkv[2 * C:3 * C][csl].T),
            "woT": np.ascontiguousarray(w_out[:, csl].T),
            "bias": (np.ascontiguousarray(
                b_out.reshape(8, 128).T.astype(np.float32))
                     if hg == 0 else np.zeros((128, 8), np.float32)),
            "masks": masks,
        }
        in_maps.append(im)
    return in_maps


def gather_outputs(results):
    """8 per-core {'yT': [C,T]} -> full [B,T,C]."""
    y = np.empty((B, T, C), np.float32)
    for b in range(B):
        acc = (results[2 * b]["yT"].astype(np.float32)
               + results[2 * b + 1]["yT"].astype(np.float32))
        y[b] = acc.T
    return y


def kernel(**inputs):
    from concourse.bass_utils import run_bass_kernel_spmd
    if "nc" not in _CACHE:
        _CACHE["nc"] = build_nc()
    nc = _CACHE["nc"]
    in_maps = shard_inputs(inputs["x"], inputs["w_qkv"],
                           inputs["w_out"], inputs["b_out"])
    res = run_bass_kernel_spmd(nc, in_maps, list(range(N_CORES)))
    return gather_outputs(res.results)


# revision 4
# speedup vs baseline: 1.4013x; 1.4013x over previous
"""Causal multi-head attention (B=4, T=2048, C=1024, H=16) on 8 TRN2 cores.

Sharding: batch (4) x head-group (2 groups of 8 heads) -> 8 shards, one per
core. Each core computes QKV projections for its 8 heads, causal flash-style
attention, and a Megatron row-parallel slice of the output projection; the
host sums the two head-group partial outputs per batch element.

v6: all matmuls bf16 (fp32 psum accum). Attention interleaves the two heads
of each pair so consecutive S matmuls occupy disjoint PE row groups (rows
0-63 / 64-127, K=64) and run concurrently. The Q/K projections for head pair
hp+1 are emitted as parcels inside attention(hp)'s loop, filling the PE while
the ACT engine grinds exp - a dedicated 1-bank psum pool (freed by moving
the softmax-denominator broadcast from a PE matmul to gpsimd
partition_broadcast) feeds them. K stays resident in SBUF (no DRAM spill).
Diagonal staircase packs to 1280 columns; the 256-wide j2 block uses a
per-head bufs=1 psum tile so the two heads' row-group-concurrent matmuls
never drain into one bank simultaneously (that crashes the device).
Phase-3 bias rides ACT.

Self-contained: hardcodes shapes from the problem spec; no file reads.
"""
import sys
sys.path.insert(0, '/opt/trn_rl_repo')
import numpy as np

B, T, C = 4, 2048, 1024
H, D = 16, 64
N_CORES = 8
HPC = 8        # heads per core
HP = 4         # head pairs per core
KB = 16        # 128-row key tiles per sequence
NQSB = 4       # 512-column query superblocks
CI = 8         # 128-row contraction tiles over C
VW = 66        # V_aug stride per head (64 V + 1 ones + 1 pad)

QOFF = (0, 128, 256, 384)
POFF = (0, 512, 1024, 896)
PTW = 1280     # packed staircase width (j0 512 + j1 384 + j3 128 + j2 256)

_CACHE = {}


def build_nc(iters=1):
    import contextlib
    import concourse.tile as tile
    from concourse import bacc, mybir

    F32 = mybir.dt.float32
    BF16 = mybir.dt.bfloat16
    EXP = mybir.ActivationFunctionType.Exp
    IDENT = mybir.ActivationFunctionType.Identity

    nc = bacc.Bacc("TRN2", target_bir_lowering=False, debug=False)

    xT_d = nc.dram_tensor("xT", [C, T], BF16, kind="ExternalInput")
    wqT_d = nc.dram_tensor("wqT", [C, 512], BF16, kind="ExternalInput")
    wkT_d = nc.dram_tensor("wkT", [C, 512], BF16, kind="ExternalInput")
    wvT_d = nc.dram_tensor("wvT", [C, 512], BF16, kind="ExternalInput")
    woT_d = nc.dram_tensor("woT", [512, C], BF16, kind="ExternalInput")
    bias_d = nc.dram_tensor("bias", [128, 8], F32, kind="ExternalInput")
    mask_d = nc.dram_tensor("masks", [128, PTW], BF16, kind="ExternalInput")
    yT_d = nc.dram_tensor("yT", [C, T], BF16, kind="ExternalOutput")

    with tile.TileContext(nc) as tc:
        def emit():
            with contextlib.ExitStack() as es:
                const = es.enter_context(tc.tile_pool(name="const", bufs=1))
                qtp = es.enter_context(tc.tile_pool(name="qt", bufs=1))
                ctxp = es.enter_context(tc.tile_pool(name="ctx", bufs=1))
                ktp = es.enter_context(tc.tile_pool(name="ktp", bufs=1))
                vp = es.enter_context(tc.tile_pool(name="vsb", bufs=1))
                xtp = es.enter_context(tc.tile_pool(name="xt", bufs=1))
                wqp = es.enter_context(tc.tile_pool(name="wq", bufs=2))
                wkp = es.enter_context(tc.tile_pool(name="wk", bufs=2))
                maskp = es.enter_context(tc.tile_pool(name="maskp", bufs=1))
                wop = es.enter_context(tc.tile_pool(name="wo", bufs=1))
                qkp = es.enter_context(
                    tc.tile_pool(name="qkp", bufs=1, space="PSUM"))

                ones_f = const.tile([128, 64], F32)
                nc.any.memset(ones_f[:], 1.0)
                ones_r = const.tile([128, 64], BF16)
                nc.vector.tensor_copy(ones_r[:], ones_f[:])
                ones16_f = const.tile([128, 16], F32)
                nc.any.memset(ones16_f[:], 1.0)
                ones16_r = const.tile([128, 16], BF16)
                nc.vector.tensor_copy(ones16_r[:], ones16_f[:])
                bias_sb = const.tile([128, 8], F32)
                mask_sb = maskp.tile([128, PTW], BF16)

                qt_sb, ctx_sb, kt_sb, v_sb, wo_sb = [], [], [], [], []
                for hp in range(HP):
                    qt_sb.append(qtp.tile([128, T], BF16, tag=f"qt{hp}",
                                          name=f"qt{hp}"))
                    ctx_sb.append(ctxp.tile([128, T], BF16, tag=f"ctx{hp}",
                                            name=f"ctx{hp}"))
                    kt_sb.append(ktp.tile([128, T], BF16, tag=f"kt{hp}",
                                          name=f"kt{hp}"))
                    w_ = wop.tile([128, C], BF16, tag=f"wo{hp}",
                                  name=f"wo{hp}")
                    wo_sb.append(w_)
                for kb in range(KB):
                    v_sb.append(vp.tile([128, HPC * VW], BF16, tag=f"v{kb}",
                                        name=f"v{kb}"))

                xt_sb = []
                wvp_o = es.enter_context(tc.tile_pool(name="wv", bufs=1))
                wv_sb = []
                for ci in range(CI):
                    t_ = xtp.tile([128, T], BF16, tag=f"xt{ci}")
                    nc.sync.dma_start(t_[:],
                                      xT_d.ap()[ci * 128:(ci + 1) * 128, :])
                    xt_sb.append(t_)
                    t2 = wvp_o.tile([128, 512], BF16, tag=f"wv{ci}",
                                    name="wvci")
                    nc.sync.dma_start(
                        t2[:], wvT_d.ap()[ci * 128:(ci + 1) * 128, :])
                    wv_sb.append(t2)
                for hp in range(HP):
                    nc.sync.dma_start(
                        wo_sb[hp][:],
                        woT_d.ap()[hp * 128:(hp + 1) * 128, :])
                nc.sync.dma_start(mask_sb[:], mask_d.ap())
                nc.sync.dma_start(bias_sb[:], bias_d.ap())

                wq_sb = [None] * HP
                wk_sb = [None] * HP

                def load_weights(hp):
                    fsl = slice(hp * 128, (hp + 1) * 128)
                    wq_sb[hp], wk_sb[hp] = [], []
                    for ci in range(CI):
                        tq = wqp.tile([128, 128], BF16, tag=f"wqs{ci}",
                                      name="wqci")
                        nc.sync.dma_start(
                            tq[:], wqT_d.ap()[ci * 128:(ci + 1) * 128, fsl])
                        wq_sb[hp].append(tq)
                        tk = wkp.tile([128, 128], BF16, tag=f"wks{ci}",
                                      name="wkci")
                        nc.sync.dma_start(
                            tk[:], wkT_d.ap()[ci * 128:(ci + 1) * 128, fsl])
                        wk_sb[hp].append(tk)

                def proj_parcels(hp):
                    """Generator: 8 parcels, each one Q or K projection
                    accumulation group (8 matmuls + DVE copy-out)."""
                    for tj in range(NQSB):
                        for which in range(2):
                            yield (hp, tj, which)

                def emit_parcel(p):
                    hp, tj, which = p
                    tsl = slice(tj * 512, (tj + 1) * 512)
                    w_sb = wq_sb[hp] if which == 0 else wk_sb[hp]
                    dst = qt_sb[hp] if which == 0 else kt_sb[hp]
                    ps_ = qkp.tile([128, 512], F32, tag="qk", name="qk")
                    for ci in range(CI):
                        nc.tensor.matmul(
                            ps_[:], w_sb[ci][:], xt_sb[ci][:, tsl],
                            start=(ci == 0), stop=(ci == CI - 1),
                            skip_group_check=True)
                    nc.vector.tensor_copy(dst[:, tsl], ps_[:])

                # ---------------- phase 1a: V projections ----------------
                with contextlib.ExitStack() as p1a:
                    vps = p1a.enter_context(
                        tc.tile_pool(name="vps", bufs=4, space="PSUM"))
                    for ti in range(KB):
                        ps_ = vps.tile([128, 512], F32)
                        for ci in range(CI):
                            nc.tensor.matmul(
                                ps_[:],
                                xt_sb[ci][:, ti * 128:(ti + 1) * 128],
                                wv_sb[ci][:],
                                start=(ci == 0), stop=(ci == CI - 1),
                                skip_group_check=True)
                        sv = v_sb[ti][:].rearrange("p (h w) -> p h w", w=VW)
                        nc.vector.tensor_copy(
                            sv[:, :, 64:66],
                            ones16_r[:].rearrange("p (h w) -> p h w", w=2))
                        nc.vector.tensor_copy(
                            sv[:, :, 0:64],
                            ps_[:].rearrange("p (h w) -> p h w", w=64))

                # -------- projections for head pair 0 (sequential) --------
                load_weights(0)
                load_weights(1)
                for p in proj_parcels(0):
                    emit_parcel(p)

                # ---------------- phase 2: attention ----------------
                with contextlib.ExitStack() as p2:
                    ptp = p2.enter_context(tc.tile_pool(name="pt", bufs=4))
                    rrp = p2.enter_context(tc.tile_pool(name="rr", bufs=2))
                    rawp = p2.enter_context(tc.tile_pool(name="raw",
                                                          bufs=3))
                    tmpp = p2.enter_context(tc.tile_pool(name="tmp", bufs=2))
                    sps0 = p2.enter_context(
                        tc.tile_pool(name="sps0", bufs=1, space="PSUM"))
                    sps1 = p2.enter_context(
                        tc.tile_pool(name="sps1", bufs=1, space="PSUM"))
                    spbp = p2.enter_context(
                        tc.tile_pool(name="spb", bufs=1, space="PSUM"))
                    pvps0 = p2.enter_context(
                        tc.tile_pool(name="pvps0", bufs=1, space="PSUM"))
                    pvps1 = p2.enter_context(
                        tc.tile_pool(name="pvps1", bufs=1, space="PSUM"))

                    PSL = (slice(0, 64), slice(64, 128))
                    for hp in range(HP):
                        kt, qt = kt_sb[hp], qt_sb[hp]
                        if hp + 1 < HP:
                            if hp + 2 < HP:
                                load_weights(hp + 2)
                            parcels = proj_parcels(hp + 1)
                        else:
                            parcels = iter(())

                        def fill():
                            p = next(parcels, None)
                            if p is not None:
                                emit_parcel(p)

                        for qsb in range(NQSB):
                            qbase = qsb * 512
                            qsl = slice(qbase, qbase + 512)
                            n_full = 4 * qsb
                            vsl = [slice((2 * hp + hl) * VW,
                                         (2 * hp + hl) * VW + 65)
                                   for hl in range(2)]
                            pv = [pvps0.tile([128, 512], F32, tag="pv0",
                                             name="pv0"),
                                  pvps1.tile([128, 512], F32, tag="pv1",
                                             name="pv1")]
                            first = [True, True]
                            for kbp in range(n_full // 2):
                                kb0, kb1 = 2 * kbp, 2 * kbp + 1
                                sp = [sps0.tile([128, 1024], F32, tag="sp0",
                                                name="sp0"),
                                      sps1.tile([128, 1024], F32, tag="sp1",
                                                name="sp1")]
                                for u, kb in enumerate((kb0, kb1)):
                                    for hl in range(2):
                                        nc.tensor.matmul(
                                            sp[hl][:, u * 512:(u + 1) * 512],
                                            kt[PSL[hl],
                                               kb * 128:(kb + 1) * 128],
                                            qt[PSL[hl], qsl],
                                            start=True, stop=True,
                                            skip_group_check=True)
                                fill()
                                pt = [None, None]
                                for hl in range(2):
                                    pt[hl] = ptp.tile([128, PTW], BF16,
                                                      tag="pt", name="pt")
                                    nc.scalar.activation(
                                        pt[hl][:, 0:1024], sp[hl][:],
                                        EXP, scale=0.125)
                                for u, kb in enumerate((kb0, kb1)):
                                    for hl in range(2):
                                        nc.tensor.matmul(
                                            pv[hl][0:65, :],
                                            v_sb[kb][:, vsl[hl]],
                                            pt[hl][:, u * 512:(u + 1) * 512],
                                            start=first[hl], stop=False,
                                            skip_group_check=True)
                                        first[hl] = False
                            # diagonal staircase
                            sp_a = [sps0.tile([128, 1024], F32, tag="sp0",
                                              name="spa0"),
                                    sps1.tile([128, 1024], F32, tag="sp1",
                                              name="spa1")]
                            for j in (0, 1, 3):
                                kb = n_full + j
                                n_ = 512 - QOFF[j]
                                for hl in range(2):
                                    nc.tensor.matmul(
                                        sp_a[hl][:, POFF[j]:POFF[j] + n_],
                                        kt[PSL[hl],
                                           kb * 128:(kb + 1) * 128],
                                        qt[PSL[hl],
                                           qbase + QOFF[j]:qbase + 512],
                                        start=True, stop=True,
                                        skip_group_check=True)
                            fill()
                            pt = [None, None]
                            for hl in range(2):
                                # per-head spb (bufs=1, same tag): head 1's
                                # write serializes behind head 0's exp read,
                                # so two row-group-concurrent matmuls never
                                # drain into this bank at the same time
                                spb = spbp.tile([128, 256], F32, tag="spb",
                                                name="spb")
                                kb2 = n_full + 2
                                nc.tensor.matmul(
                                    spb[:],
                                    kt[PSL[hl], kb2 * 128:(kb2 + 1) * 128],
                                    qt[PSL[hl],
                                       qbase + QOFF[2]:qbase + 512],
                                    start=True, stop=True,
                                    skip_group_check=True)
                                p_ = ptp.tile([128, PTW], BF16, tag="pt",
                                              name="pt")
                                nc.scalar.activation(p_[:, 0:1024],
                                                     sp_a[hl][:],
                                                     EXP, scale=0.125)
                                nc.scalar.activation(p_[:, 1024:PTW],
                                                     spb[:],
                                                     EXP, scale=0.125)
                                nc.vector.tensor_mul(p_[:], p_[:],
                                                     mask_sb[:])
                                pt[hl] = p_
                                for j in (0, 1, 3, 2):
                                    kb = n_full + j
                                    n_ = 512 - QOFF[j]
                                    nc.tensor.matmul(
                                        pv[hl][0:65, QOFF[j]:512],
                                        v_sb[kb][:, vsl[hl]],
                                        pt[hl][:, POFF[j]:POFF[j] + n_],
                                        start=first[hl], stop=(j == 2),
                                        skip_group_check=True)
                                    first[hl] = False
                            # normalize: ctx = pv[0:64] / pv[64]; denominator
                            # reciprocal broadcast across partitions on gpsimd
                            for hl in range(2):
                                rr = rrp.tile([65, 512], BF16, tag="rr",
                                              name="rr")
                                with nc.allow_low_precision("softmax denom"):
                                    nc.vector.reciprocal(rr[64:65, :],
                                                         pv[hl][64:65, :])
                                # denominator broadcast via PE, time-sharing
                                # the projection-parcel psum bank
                                bc = qkp.tile([128, 512], F32, tag="qk",
                                              name="bc")
                                nc.tensor.matmul(bc[0:64, :],
                                                 ones_r[64:65, :],
                                                 rr[64:65, :],
                                                 start=True, stop=True,
                                                 skip_group_check=True)
                                raw = rawp.tile([64, 512], F32, tag="raw",
                                                name="raw")
                                nc.vector.tensor_copy(raw[:],
                                                      pv[hl][0:64, :])
                                if hl == 0:
                                    nc.vector.tensor_mul(
                                        ctx_sb[hp][0:64, qsl],
                                        raw[:], bc[0:64, :])
                                else:
                                    tmp = tmpp.tile([64, 512], BF16,
                                                    tag="tmp", name="tmp")
                                    nc.vector.tensor_mul(tmp[:], raw[:],
                                                         bc[0:64, :])
                                    nc.sync.dma_start(
                                        ctx_sb[hp][64:128, qsl], tmp[:])
                        for p in parcels:
                            emit_parcel(p)

                    # -------------- phase 3: output projection --------------
                    with contextlib.ExitStack() as p3:
                        yp = p3.enter_context(tc.tile_pool(name="y", bufs=3))
                        for oi in range(8):
                            osl = slice(oi * 128, (oi + 1) * 128)
                            for tj in range(NQSB):
                                tsl = slice(tj * 512, (tj + 1) * 512)
                                yps = pvps0 if (oi * NQSB + tj) % 2 == 0 \
                                    else pvps1
                                ps_ = yps.tile([128, 512], F32,
                                               tag="pv0" if yps is pvps0
                                               else "pv1", name="yacc")
                                for hp in range(HP):
                                    nc.tensor.matmul(
                                        ps_[:], wo_sb[hp][:, osl],
                                        ctx_sb[hp][:, tsl],
                                        start=(hp == 0), stop=(hp == HP - 1),
                                        skip_group_check=True)
                                y_ = yp.tile([128, 512], BF16)
                                nc.scalar.activation(
                                    y_[:], ps_[:], IDENT,
                                    bias=bias_sb[:, oi:oi + 1])
                                nc.sync.dma_start(yT_d.ap()[osl, tsl], y_[:])

        if iters == 1:
            emit()
        else:
            with tc.For_i(0, iters, 1):
                emit()
    nc.compile()
    return nc


def make_masks():
    """Packed staircase mask [128, PTW]: pt col POFF[j] + (q - QOFF[j])
    holds causal keep-bit for key row k = 128*j + k_local vs query q."""
    m = np.zeros((128, PTW), np.float32)
    k = np.arange(128)[:, None]
    for j in range(4):
        q = np.arange(QOFF[j], 512)[None, :]
        m[:, POFF[j]:POFF[j] + 512 - QOFF[j]] = (q >= 128 * j + k)
    return m


def shard_inputs(x, w_qkv, w_out, b_out):
    """Full inputs -> list of 8 per-core input dicts."""
    import ml_dtypes
    bf16 = ml_dtypes.bfloat16
    x = np.asarray(x, dtype=np.float32).astype(bf16)
    w_qkv = np.asarray(w_qkv, dtype=np.float32).astype(bf16)
    w_out = np.asarray(w_out, dtype=np.float32).astype(bf16)
    b_out = np.asarray(b_out, dtype=np.float32)
    masks = make_masks().astype(bf16)
    in_maps = []
    for c in range(N_CORES):
        b, hg = c // 2, c % 2
        h0 = hg * HPC
        csl = slice(h0 * D, (h0 + HPC) * D)
        im = {
            "xT": np.ascontiguousarray(x[b].T),
            "wqT": np.ascontiguousarray(w_qkv[0 * C:1 * C][csl].T),
            "wkT": np.ascontiguousarray(w_qkv[1 * C:2 * C][csl].T),
            "wvT": np.ascontiguousarray(w_qkv[2 * C:3 * C][csl].T),
            "woT": np.ascontiguousarray(w_out[:, csl].T),
            "bias": (np.ascontiguousarray(
                b_out.reshape(8, 128).T.astype(np.float32))
                     if hg == 0 else np.zeros((128, 8), np.float32)),
            "masks": masks,
        }
        in_maps.append(im)
    return in_maps


def gather_outputs(results):
    """8 per-core {'yT': [C,T]} -> full [B,T,C]."""
    y = np.empty((B, T, C), np.float32)
    for b in range(B):
        acc = (results[2 * b]["yT"].astype(np.float32)
               + results[2 * b + 1]["yT"].astype(np.float32))
        y[b] = acc.T
    return y


def kernel(**inputs):
    from concourse.bass_utils import run_bass_kernel_spmd
    if "nc" not in _CACHE:
        _CACHE["nc"] = build_nc()
    nc = _CACHE["nc"]
    in_maps = shard_inputs(inputs["x"], inputs["w_qkv"],
                           inputs["w_out"], inputs["b_out"])
    res = run_bass_kernel_spmd(nc, in_maps, list(range(N_CORES)))
    return gather_outputs(res.results)
